# revision 1
# baseline (speedup 1.0000x reference)
"""Trainium2 Bass kernel for nn_DecoderRNN (autoregressive LSTM decoder).

Strategy:
  - Pure data parallelism: batch 8192 -> 1024 per core across 8 NeuronCores.
  - Feature-major layout on chip: h^T, c^T are [H=128 partitions, B_local].
    gates^T = W^T blocks (stationary) @ activations (moving), so the
    elementwise LSTM cell update produces h^T directly in the layout the
    next step's matmul needs -- no per-step transposes.
  - Output projection folded into the recurrent weights (W~_hh = W_hh +
    W_ih[:,0:1] @ W_out); biases ride in the matmul via a ones-row in the
    z tile. Step 0 uses unfolded weights with x supplied via the z tile.
  - ACT (scalar engine) is the bottleneck: 5 transcendental passes over
    [128,1024] per step = 4.27us/step of column time at 1.2GHz, plus
    ~185ns fixed cost per activation instruction. The schedule targets
    exactly 8 ACT ops/step (sFI, G, sO, ct per half) in the stream order
      sFI0 G0 sO0 sFI1 G1 ct0 sO1 ct1
    which keeps every op's inputs ready >= its slot start and closes the
    per-half recurrence cycle (evict -> cell -> tanh(c) -> h -> h-side
    matmul -> next evict) in exactly the 6.0us/step ACT busy time.
  - Gate order [f, i, g, o]: f,i share a 2-bank PSUM tile (one sigmoid
    evict); g and o are 1-bank tiles. tanh(c) is one [128,512] op per
    half (not quartered: fewer ACT ops wins over finer pipelining).
  - Gate evictions write bf16: i*g runs in the DVE 2x packed mode, and
    h = o*tanh(c) is all-bf16 (h feeds the matmuls as bf16 moving data,
    same 1 cycle/row as f32r). c and f*c stay fp32 for accuracy.
  - All three cell ops (f*c, i*g, add) run on DVE -- GPSIMD's 0.42x
    multiply efficiency makes it too slow for the critical path; it only
    gets the out-row PSUM->SBUF copies.
  - z-side matmuls for step t+1 are pre-issued during step t; h-side
    matmuls are emitted f,i,g,o so the sFI evict unblocks after two mms.
  - Out rows: step t / half b lands on PSUM partition 32*b + t%32 via
    shifted W_out column blocks, so 64 rows accumulate in one PSUM bank
    and evict once per 32 steps.
  - PSUM budget (8 banks): FI 2bufs x 2 + G 2bufs x 1 + O 1buf x 1 +
    po32 1 = 8.
"""

import os
import sys

for _p in ("/opt/trn_rl_repo", "/root/.axon_site/_ro/trn_rl_repo"):
    if os.path.isdir(_p) and _p not in sys.path:
        sys.path.insert(0, _p)

from contextlib import ExitStack

import numpy as np

import concourse.bass as bass  # noqa: F401  (registers types)
import concourse.mybir as mybir
import concourse.tile as tile
from concourse import bacc
from concourse.bass_utils import run_bass_kernel_spmd

NCORES = 8
B, T, F, H, P = 8192, 128, 63, 128, 64
BL = B // NCORES      # 1024 rows per core
I = 2 + F             # 64 LSTM input features + 1 ones-row for bias
G4 = 4 * H            # 512 gate rows
NH = 2                # batch halves (moving-dim chunks of 512)
NW = BL // NH         # 512

_f32 = mybir.dt.float32
_f32r = mybir.dt.float32r
_bf16 = mybir.dt.bfloat16

_CACHE: dict = {}


def _build():
    nc = bacc.Bacc("TRN2", target_bir_lowering=False, debug=False)
    AF = mybir.ActivationFunctionType

    zt_d = nc.dram_tensor("zt", [P, I, BL], _f32r, kind="ExternalInput")
    # packed inputs, one per DGE queue so the four prologue transfers run
    # in parallel: wz: [wz0 | wzf] f32r; wh0h: [wh0 | h0] bf16;
    # wrest: [whf | wo | c0] bf16. z_0 comes straight from zt.
    # weight layouts: columns are gate rows permuted to [f, i, g, o]
    wz_d = nc.dram_tensor("wzt", [I, 2 * G4], _f32r, kind="ExternalInput")
    wh0h_d = nc.dram_tensor("wh0ht", [H, G4 + BL], _bf16, kind="ExternalInput")
    wrest_d = nc.dram_tensor("wrestt", [H, G4 + H + BL], _bf16,
                             kind="ExternalInput")
    out_d = nc.dram_tensor("out", [P, BL], _f32, kind="ExternalOutput")

    with ExitStack() as ctx:
        tc = ctx.enter_context(tile.TileContext(nc))
        const = ctx.enter_context(tc.tile_pool(name="const", bufs=1))
        zp = ctx.enter_context(tc.tile_pool(name="z", bufs=4))
        hp = ctx.enter_context(tc.tile_pool(name="h", bufs=2))
        cp = ctx.enter_context(tc.tile_pool(name="c", bufs=2))
        gp = ctx.enter_context(tc.tile_pool(name="g", bufs=3))
        tp = ctx.enter_context(tc.tile_pool(name="t", bufs=3))
        op = ctx.enter_context(tc.tile_pool(name="osb", bufs=3))
        # PSUM budget (8 banks): FI 2x2 + G 2x1 + O 1x1 + po32 1 = 8
        psfi = ctx.enter_context(tc.tile_pool(name="psfi", bufs=2, space="PSUM"))
        psg = ctx.enter_context(tc.tile_pool(name="psg", bufs=1, space="PSUM"))
        pso = ctx.enter_context(tc.tile_pool(name="pso", bufs=2, space="PSUM"))
        pspo = ctx.enter_context(tc.tile_pool(name="pspo", bufs=1, space="PSUM"))

        # PE warmup: the tensor engine's clock ramps with sustained use
        # (p-state model: 0.65 -> 1.2 -> 2.4 GHz after 3us busy). Dummy
        # matmuls during the input-DMA window mean the real step-0 matmuls
        # start at full clock instead of 1.54 ns/col.
        wrm = tp.tile([H, H], _bf16, tag="wrm")
        nc.vector.memset(wrm[:], 0.0)
        wps = pspo.tile([H, H], _f32, tag="po32", name="warmup_ps")
        for _w in range(44):
            nc.tensor.matmul(wps[:], wrm[:], wrm[:], start=True, stop=True)

        # step-0-critical tensors first so the pipeline fills ASAP; four
        # packed transfers on four different DGE queues run in parallel.
        wzt = const.tile([I, 2 * G4], _f32r, tag="wzt")
        nc.sync.dma_start(wzt[:], wz_d[:])
        wz0 = wzt[:, 0:G4]
        wzf = wzt[:, G4 : 2 * G4]
        zt0 = zp.tile([I, BL], _f32r, tag="z", name="z0")
        nc.scalar.dma_start(zt0[:], zt_d[0, :, :])
        wh0ht = const.tile([H, G4 + BL], _bf16, tag="wh0ht")
        nc.gpsimd.dma_start(wh0ht[:], wh0h_d[:])
        wh0 = wh0ht[:, 0:G4]
        h_prev = wh0ht[:, G4 : G4 + BL]
        wrestt = const.tile([H, G4 + H + BL], _bf16, tag="wrestt")
        nc.sync.dma_start(wrestt[:], wrest_d[:])
        whf = wrestt[:, 0:G4]
        wo = wrestt[:, G4 : G4 + H]
        c_prev = wrestt[:, G4 + H : G4 + H + BL]

        def z_mms(t, zt, ps):
            """z-side (and bias) matmul contributions for step t; emitted
            during step t-1, they run while the PE waits for h_t. For t=0
            the matching h-side matmuls are interleaved per gate block so
            the first sigmoid evict isn't stuck behind all 16 z-matmuls on
            the cold (p-state-throttled) PE."""
            wz = wz0 if t == 0 else wzf
            nq = 2 if t == 0 else 1
            qw = NW // nq
            for half in range(NH):
                psFI = psfi.tile([H, 2 * NW], _f32, tag="fi",
                                 name=f"psFI{t}_{half}")
                psG = psg.tile([H, NW], _f32, tag="g", name=f"psG{t}_{half}")
                psO = pso.tile([H, NW], _f32, tag="o", name=f"psO{t}_{half}")
                ps[(t, half)] = (psFI, psG, psO)
                for j in range(2):      # f, i blocks
                    for q in range(nq):
                        js = slice(j * NW + q * qw, j * NW + (q + 1) * qw)
                        qs = slice(half * NW + q * qw, half * NW + (q + 1) * qw)
                        nc.tensor.matmul(psFI[:, js], wz[:, j * H : (j + 1) * H],
                                         zt[:, qs], start=(q == 0), stop=False)
                if t == 0:
                    h_mms_fi(t, half, psFI)
                for q in range(nq):
                    qs = slice(half * NW + q * qw, half * NW + (q + 1) * qw)
                    qj = slice(q * qw, (q + 1) * qw)
                    nc.tensor.matmul(psG[:, qj], wz[:, 2 * H : 3 * H], zt[:, qs],
                                     start=(q == 0), stop=False)
                    nc.tensor.matmul(psO[:, qj], wz[:, 3 * H : 4 * H], zt[:, qs],
                                     start=(q == 0), stop=False)
                if t == 0:
                    h_mms_go(t, half, psG, psO)

        def h_mms_fi(t, half, psFI):
            """h-side f,i matmuls in 256-col quarters (h lands in quarter
            chunks from the split h-mul, so the first mms start early and
            the sFI evict unblocks sooner)."""
            wh = wh0 if t == 0 else whf
            hw_ = NW // 2
            for q in range(2):
                for j in range(2):
                    js = slice(j * NW + q * hw_, j * NW + (q + 1) * hw_)
                    qs = slice(half * NW + q * hw_, half * NW + (q + 1) * hw_)
                    nc.tensor.matmul(psFI[:, js], wh[:, j * H : (j + 1) * H],
                                     h_prev[:, qs], start=False, stop=True)

        def h_mms_go(t, half, psG, psO):
            wh = wh0 if t == 0 else whf
            cs = slice(half * NW, (half + 1) * NW)
            nc.tensor.matmul(psG[:], wh[:, 2 * H : 3 * H], h_prev[:, cs],
                             start=False, stop=True)
            nc.tensor.matmul(psO[:], wh[:, 3 * H : 4 * H], h_prev[:, cs],
                             start=False, stop=True)

        ps: dict = {}
        z_mms(0, zt0, ps)

        po32: dict = {}

        _PO_GROUPS = {}
        for _g0, _glen in ((0, 32), (32, 31), (63, 1)):
            for _t in range(_g0, _g0 + _glen):
                _PO_GROUPS[_t] = (_g0, _glen)

        def emit_po(tp_, h_tile):
            g0, glen = _PO_GROUPS[tp_]
            j = tp_ - g0
            if j == 0:
                po32[0] = pspo.tile([64, NW], _f32, tag="po32",
                                    name=f"po32_{tp_}")
            for half in range(NH):
                cs = slice(half * NW, (half + 1) * NW)
                blk = 63 - (half * 32 + j)
                nc.tensor.matmul(po32[0][:], wo[:, blk : blk + 64],
                                 h_tile[:, cs],
                                 start=(j == 0 and half == 0),
                                 stop=(j == glen - 1 and half == NH - 1))
            if j == glen - 1:
                orow32 = op.tile([64, NW], _f32, tag="orow", name=f"orow{tp_}")
                nc.vector.tensor_copy(orow32[:], po32[0][:])
                if glen == 1:
                    nc.sync.dma_start(out_d[g0 : g0 + 1, :],
                                      orow32[0:64:32, :])
                else:
                    for half in range(NH):
                        cs = slice(half * NW, (half + 1) * NW)
                        nc.sync.dma_start(out_d[g0 : g0 + glen, cs],
                                          orow32[32 * half : 32 * half + glen, :])

        prev = None  # (t, h_tile) pending out-projection
        for t in range(P):
            h_new = hp.tile([H, BL], _bf16, tag="h", name=f"h{t}")
            c_new = cp.tile([H, BL], _bf16, tag="c", name=f"c{t}")
            gFI = [None, None]
            gG = [None, None]
            gO = [None, None]
            ct = [None, None]

            def evict_fi(half):
                psFI, _, _ = ps[(t, half)]
                gFI[half] = gp.tile([H, 2 * NW], _bf16, tag="gFI",
                                    name=f"gFI{t}_{half}")
                nc.scalar.activation(gFI[half][:], psFI[:], AF.Sigmoid)

            def evict_g(half):
                _, psG, _ = ps[(t, half)]
                gG[half] = gp.tile([H, NW], _bf16, tag="gG",
                                   name=f"gG{t}_{half}")
                nc.scalar.activation(gG[half][:], psG[:], AF.Tanh)

            def evict_o(half):
                _, _, psO = ps[(t, half)]
                gO[half] = gp.tile([H, NW], _bf16, tag="gO",
                                   name=f"gO{t}_{half}")
                nc.scalar.activation(gO[half][:], psO[:], AF.Sigmoid)

            def cell(half):
                """c = f*c_prev + i*g, all on DVE (t1 fp32, t2 bf16 2x)."""
                cs = slice(half * NW, (half + 1) * NW)
                f_s = gFI[half][:, 0:NW]
                i_s = gFI[half][:, NW : 2 * NW]
                t1 = tp.tile([H, NW], _bf16, tag="t1", name=f"t1_{t}_{half}")
                nc.vector.tensor_mul(t1[:], f_s, c_prev[:, cs])
                t2 = tp.tile([H, NW], _bf16, tag="t2", name=f"t2_{t}_{half}")
                nc.vector.tensor_mul(t2[:], i_s, gG[half][:])
                nc.vector.tensor_add(c_new[:, cs], t1[:], t2[:])

            def tanh_c(half):
                cs = slice(half * NW, (half + 1) * NW)
                ct[half] = tp.tile([H, NW], _bf16, tag="ct",
                                   name=f"ct{t}_{half}")
                nc.scalar.activation(ct[half][:], c_new[:, cs], AF.Tanh)

            def h_mul(half):
                """h = o * tanh(c) in two 256-col chunks so the next step's
                f,i matmuls can start on the first chunk early."""
                hw_ = NW // 2
                for q in range(2):
                    qs = slice(half * NW + q * hw_, half * NW + (q + 1) * hw_)
                    qq = slice(q * hw_, (q + 1) * hw_)
                    nc.vector.tensor_mul(h_new[:, qs], gO[half][:, qq],
                                         ct[half][:, qq])

            # ACT priority order: sFI0 G0 sFI1 G1 sO0 ct0 sO1 ct1.
            # G1 early pulls half-1's i*g / c-add forward on the DVE, so the
            # h0 mul later finds the DVE free and the return path (ct0 ->
            # h0 -> f,i matmuls -> sFI0 of step t+1) fits inside the sO1+ct1
            # ACT slots.
            if t > 0:
                psFI0, psG0, psO0 = ps[(t, 0)]
                h_mms_fi(t, 0, psFI0)
                h_mms_go(t, 0, psG0, psO0)
            evict_fi(0)
            evict_g(0)
            cell(0)
            if t > 0:
                psFI1, psG1, psO1 = ps[(t, 1)]
                h_mms_fi(t, 1, psFI1)
                h_mms_go(t, 1, psG1, psO1)
            evict_fi(1)
            evict_g(1)
            evict_o(0)
            tanh_c(0)
            cell(1)
            h_mul(0)
            evict_o(1)
            tanh_c(1)
            h_mul(1)
            ps.pop((t, 0))
            ps.pop((t, 1))
            if t + 1 < P:
                zt = zp.tile([I, BL], _f32r, tag="z", name=f"z{t + 1}")
                nc.sync.dma_start(zt[:], zt_d[t + 1, :, :])
                z_mms(t + 1, zt, ps)
            if prev is not None:
                emit_po(prev[0], prev[1])
            prev = (t, h_new)
            h_prev, c_prev = h_new, c_new
        emit_po(prev[0], prev[1])

    nc.compile()
    return nc


def _get_nc():
    if "nc" not in _CACHE:
        _CACHE["nc"] = _build()
    return _CACHE["nc"]


# gate-row permutation: PyTorch order [i,f,g,o] -> kernel order [f,i,g,o]
_PERM = np.concatenate(
    [np.arange(H, 2 * H), np.arange(0, H), np.arange(2 * H, 3 * H),
     np.arange(3 * H, 4 * H)]
)


def _prep_in_maps(x, z, h0, c0, W_ih, W_hh, b_ih, b_hh, W_out, b_out):
    f = np.float32
    Wihp = W_ih[_PERM]                                   # (512, 64)
    Whhp = W_hh[_PERM]                                   # (512, 128)
    Whfp = Whhp + Wihp[:, 0:1] @ W_out                   # fold out-projection
    b0 = (b_ih + b_hh)[_PERM].astype(f)
    bf = (b0 + Wihp[:, 0] * b_out[0]).astype(f)

    wz0t = np.concatenate([Wihp.T, b0[None, :]], axis=0).astype(f)   # (65, 512)
    wzft = np.concatenate([Wihp.T, bf[None, :]], axis=0).astype(f)   # (65, 512)
    whh0t = np.ascontiguousarray(Whhp.T, dtype=f)                    # (128, 512)
    whhft = np.ascontiguousarray(Whfp.T, dtype=f)                    # (128, 512)
    woutt = np.zeros((H, H), dtype=f)
    woutt[:, 63] = W_out[0]

    import ml_dtypes
    in_maps = []
    for m in range(NCORES):
        sl = slice(m * BL, (m + 1) * BL)
        z_aug = np.empty((P, I, BL), dtype=f)
        z_aug[:, 0, :] = 0.0
        z_aug[0, 0, :] = x[sl, -1, 0]
        z_aug[:, 1:-1, :] = np.transpose(z[sl, T - P :, :], (1, 2, 0))
        z_aug[:, -1, :] = 1.0
        wh0h = np.concatenate([whh0t, h0[0, sl, :].T], axis=1)   # (128, 1536)
        wrest = np.concatenate(
            [whhft, woutt, c0[0, sl, :].T], axis=1)              # (128, 1664)
        in_maps.append(
            {
                "zt": np.ascontiguousarray(z_aug),
                "wzt": np.ascontiguousarray(
                    np.concatenate([wz0t, wzft], axis=1)),       # (65, 1024)
                "wh0ht": np.ascontiguousarray(wh0h).astype(ml_dtypes.bfloat16),
                "wrestt": np.ascontiguousarray(wrest).astype(ml_dtypes.bfloat16),
            }
        )
    return in_maps


def run_on_cores(inputs: dict, **spmd_kwargs):
    """Build + run; returns (full_output, BassKernelResults)."""
    inputs = {k: np.asarray(v, dtype=np.float32) for k, v in inputs.items()}
    nc = _get_nc()
    in_maps = _prep_in_maps(**inputs)
    res = run_bass_kernel_spmd(nc, in_maps, core_ids=list(range(NCORES)), **spmd_kwargs)
    outs = np.concatenate(
        [r["out"].T for r in res.results], axis=0
    )  # (8192, 64)
    outs = outs + np.float32(inputs["b_out"][0])
    return outs[:, :, None].astype(np.float32), res


def kernel(**inputs) -> np.ndarray:
    out, _ = run_on_cores(inputs)
    return out



# revision 2
# speedup vs baseline: 1.0036x; 1.0036x over previous
"""Trainium2 Bass kernel for nn_DecoderRNN (autoregressive LSTM decoder).

Strategy:
  - Pure data parallelism: batch 8192 -> 1024 per core across 8 NeuronCores.
  - Feature-major layout on chip: h^T, c^T are [H=128 partitions, B_local].
    gates^T = W^T blocks (stationary) @ activations (moving), so the
    elementwise LSTM cell update produces h^T directly in the layout the
    next step's matmul needs -- no per-step transposes.
  - Output projection folded into the recurrent weights (W~_hh = W_hh +
    W_ih[:,0:1] @ W_out); biases ride in the matmul via a ones-row in the
    z tile. Step 0 uses unfolded weights with x supplied via the z tile.
  - ACT (scalar engine) is the bottleneck: 5120 activation elements/step at
    0.833ns each + 185ns fixed cost per ACT instruction. To minimize the
    instruction count the F,I,G gates share ONE fused PSUM tile per batch
    half and evict in a single [128,1536] Sigmoid: the G tanh is computed
    as tanh(g) = 2*sigmoid(2g)-1 (G-gate weight rows pre-scaled by 2; the
    2s-1 correction is one 4x-mode tensor_scalar on DVE). That gives 6 ACT
    ops/step (sFIG x2, sO x2, tanh(c) x2) = ~5.38us vs 8 ops = 5.75us.
  - PSUM budget (8 banks): FIG fused [128,1536] = 3 banks x 2 bufs = 6,
    O [128,512] 1 bank x 1 buf (t+1 half-0 O-matmuls wait for step t's
    half-1 O eviction; PE has slack late in the step), po32 1 bank.
  - fp16 (not bf16) for all on-chip elementwise data and the recurrent
    weights: same speed everywhere (DVE 2-byte packed modes, 1 cyc/col PE
    moving data), 8x finer mantissa -- keeps the sigmoid-fold exact enough.
  - Cell math per half: tg = 2*sg-1 (tensor_scalar, 4x mode), p = i*tg,
    q = f*c_prev, c = p+q (tensor_tensor, 2x mode). DVE order: cell0,
    cell1, h_mul0, h_mul1 -- cell1 must precede h_mul0 so ACT's ct1 slot
    (last) gets c1 in time.
  - z-side matmuls for step t+1 are pre-issued during step t; h-side
    matmuls at the start of step t. F,I h-matmuls in 256-col quarters so
    the fused eviction unblocks as h chunks land.
  - Out rows: step t / half b lands on PSUM partition 32*b + t%32 via
    shifted W_out column blocks, so 64 rows accumulate in one PSUM bank
    and evict once per 32 steps.
"""

import os
import sys

for _p in ("/opt/trn_rl_repo", "/root/.axon_site/_ro/trn_rl_repo"):
    if os.path.isdir(_p) and _p not in sys.path:
        sys.path.insert(0, _p)

from contextlib import ExitStack

import numpy as np

import concourse.bass as bass  # noqa: F401  (registers types)
import concourse.mybir as mybir
import concourse.tile as tile
from concourse import bacc
from concourse.bass_utils import run_bass_kernel_spmd

NCORES = 8
B, T, F, H, P = 8192, 128, 63, 128, 64
BL = B // NCORES      # 1024 rows per core
I = 2 + F             # 64 LSTM input features + 1 ones-row for bias
G4 = 4 * H            # 512 gate rows
NH = 2                # batch halves (moving-dim chunks of 512)
NW = BL // NH         # 512

_f32 = mybir.dt.float32
_f32r = mybir.dt.float32r
_f16 = mybir.dt.float16

_CACHE: dict = {}


def _build():
    nc = bacc.Bacc("TRN2", target_bir_lowering=False, debug=False)
    AF = mybir.ActivationFunctionType
    ALU = mybir.AluOpType

    zt_d = nc.dram_tensor("zt", [P, I, BL], _f32r, kind="ExternalInput")
    # packed inputs, one per DGE queue so the four prologue transfers run
    # in parallel: wz: [wz0 | wzf] f32r; wh0h: [wh0 | h0] f16;
    # wrest: [whf | wo | c0] f16. z_0 comes straight from zt.
    # weight layouts: columns are gate rows permuted to [f, i, g, o],
    # G-gate columns pre-scaled by 2 for the sigmoid-fold.
    wz_d = nc.dram_tensor("wzt", [I, 2 * G4], _f32r, kind="ExternalInput")
    wh0h_d = nc.dram_tensor("wh0ht", [H, G4 + BL], _f16, kind="ExternalInput")
    wrest_d = nc.dram_tensor("wrestt", [H, G4 + H + BL], _f16,
                             kind="ExternalInput")
    out_d = nc.dram_tensor("out", [P, BL], _f32, kind="ExternalOutput")

    with ExitStack() as ctx:
        tc = ctx.enter_context(tile.TileContext(nc))
        const = ctx.enter_context(tc.tile_pool(name="const", bufs=1))
        zp = ctx.enter_context(tc.tile_pool(name="z", bufs=4))
        hp = ctx.enter_context(tc.tile_pool(name="h", bufs=2))
        cp = ctx.enter_context(tc.tile_pool(name="c", bufs=2))
        gp = ctx.enter_context(tc.tile_pool(name="g", bufs=3))
        tp = ctx.enter_context(tc.tile_pool(name="t", bufs=3))
        op = ctx.enter_context(tc.tile_pool(name="osb", bufs=3))
        # PSUM budget (8 banks): FIG 2bufs x 3 + O 1buf x 1 + po32 1 = 8
        psfig = ctx.enter_context(tc.tile_pool(name="psfig", bufs=2,
                                               space="PSUM"))
        pso = ctx.enter_context(tc.tile_pool(name="pso", bufs=1, space="PSUM"))
        pspo = ctx.enter_context(tc.tile_pool(name="pspo", bufs=1, space="PSUM"))

        # PE warmup: the tensor engine's clock ramps with sustained use
        # (p-state model: 0.65 -> 1.2 -> 2.4 GHz after 3us busy). Dummy
        # matmuls during the input-DMA window mean the real step-0 matmuls
        # start at full clock instead of 1.54 ns/col.
        wrm = tp.tile([H, H], _f16, tag="wrm")
        nc.vector.memset(wrm[:], 0.0)
        wps = pspo.tile([H, H], _f32, tag="po32", name="warmup_ps")
        for _w in range(44):
            nc.tensor.matmul(wps[:], wrm[:], wrm[:], start=True, stop=True)

        # step-0-critical tensors first so the pipeline fills ASAP; four
        # packed transfers on four different DGE queues run in parallel.
        wzt = const.tile([I, 2 * G4], _f32r, tag="wzt")
        nc.sync.dma_start(wzt[:], wz_d[:])
        wz0 = wzt[:, 0:G4]
        wzf = wzt[:, G4 : 2 * G4]
        zt0 = zp.tile([I, BL], _f32r, tag="z", name="z0")
        nc.scalar.dma_start(zt0[:], zt_d[0, :, :])
        wh0ht = const.tile([H, G4 + BL], _f16, tag="wh0ht")
        nc.gpsimd.dma_start(wh0ht[:], wh0h_d[:])
        wh0 = wh0ht[:, 0:G4]
        h_prev = wh0ht[:, G4 : G4 + BL]
        wrestt = const.tile([H, G4 + H + BL], _f16, tag="wrestt")
        nc.sync.dma_start(wrestt[:], wrest_d[:])
        whf = wrestt[:, 0:G4]
        wo = wrestt[:, G4 : G4 + H]
        c_prev = wrestt[:, G4 + H : G4 + H + BL]

        def h_mms_fig(t, half, psFIG):
            """h-side f,i matmuls in 256-col quarters (h lands in quarter
            chunks from the split h-mul, so the first mms start early and
            the fused eviction unblocks sooner); g full-width."""
            wh = wh0 if t == 0 else whf
            hw_ = NW // 2
            for q in range(2):
                for j in range(2):
                    js = slice(j * NW + q * hw_, j * NW + (q + 1) * hw_)
                    qs = slice(half * NW + q * hw_, half * NW + (q + 1) * hw_)
                    nc.tensor.matmul(psFIG[:, js], wh[:, j * H : (j + 1) * H],
                                     h_prev[:, qs], start=False, stop=True)
            cs = slice(half * NW, (half + 1) * NW)
            nc.tensor.matmul(psFIG[:, 2 * NW : 3 * NW], wh[:, 2 * H : 3 * H],
                             h_prev[:, cs], start=False, stop=True)

        def h_mm_o(t, half, psO):
            wh = wh0 if t == 0 else whf
            cs = slice(half * NW, (half + 1) * NW)
            nc.tensor.matmul(psO[:], wh[:, 3 * H : 4 * H], h_prev[:, cs],
                             start=False, stop=True)

        def z_mms(t, zt, ps):
            """z-side (and bias) matmul contributions for step t; emitted
            during step t-1, they run while the PE waits for h_t. For t=0
            the matching h-side matmuls are interleaved per gate block so
            the first eviction isn't stuck behind all z-matmuls on the
            cold (p-state-throttled) PE."""
            wz = wz0 if t == 0 else wzf
            nq = 2 if t == 0 else 1
            qw = NW // nq
            for half in range(NH):
                psFIG = psfig.tile([H, 3 * NW], _f32, tag="fig",
                                   name=f"psFIG{t}_{half}")
                psO = pso.tile([H, NW], _f32, tag="o", name=f"psO{t}_{half}")
                ps[(t, half)] = (psFIG, psO)
                for j in range(3):      # f, i, g blocks
                    for q in range(nq):
                        js = slice(j * NW + q * qw, j * NW + (q + 1) * qw)
                        qs = slice(half * NW + q * qw, half * NW + (q + 1) * qw)
                        nc.tensor.matmul(psFIG[:, js], wz[:, j * H : (j + 1) * H],
                                         zt[:, qs], start=(q == 0), stop=False)
                if t == 0:
                    h_mms_fig(t, half, psFIG)
                for q in range(nq):
                    qs = slice(half * NW + q * qw, half * NW + (q + 1) * qw)
                    qj = slice(q * qw, (q + 1) * qw)
                    nc.tensor.matmul(psO[:, qj], wz[:, 3 * H : 4 * H], zt[:, qs],
                                     start=(q == 0), stop=False)
                if t == 0:
                    h_mm_o(t, half, psO)

        ps: dict = {}
        z_mms(0, zt0, ps)

        po32: dict = {}

        _PO_GROUPS = {}
        for _g0, _glen in ((0, 32), (32, 31), (63, 1)):
            for _t in range(_g0, _g0 + _glen):
                _PO_GROUPS[_t] = (_g0, _glen)

        def emit_po(tp_, h_tile):
            g0, glen = _PO_GROUPS[tp_]
            j = tp_ - g0
            if j == 0:
                po32[0] = pspo.tile([64, NW], _f32, tag="po32",
                                    name=f"po32_{tp_}")
            for half in range(NH):
                cs = slice(half * NW, (half + 1) * NW)
                blk = 63 - (half * 32 + j)
                nc.tensor.matmul(po32[0][:], wo[:, blk : blk + 64],
                                 h_tile[:, cs],
                                 start=(j == 0 and half == 0),
                                 stop=(j == glen - 1 and half == NH - 1))
            if j == glen - 1:
                orow32 = op.tile([64, NW], _f32, tag="orow", name=f"orow{tp_}")
                nc.vector.tensor_copy(orow32[:], po32[0][:])
                if glen == 1:
                    nc.sync.dma_start(out_d[g0 : g0 + 1, :],
                                      orow32[0:64:32, :])
                else:
                    for half in range(NH):
                        cs = slice(half * NW, (half + 1) * NW)
                        nc.sync.dma_start(out_d[g0 : g0 + glen, cs],
                                          orow32[32 * half : 32 * half + glen, :])

        prev = None  # (t, h_tile) pending out-projection
        for t in range(P):
            h_new = hp.tile([H, BL], _f16, tag="h", name=f"h{t}")
            c_new = cp.tile([H, BL], _f16, tag="c", name=f"c{t}")
            gFIG = [None, None]
            gO = [None, None]
            ct = [None, None]

            def evict_fig(half):
                psFIG, _ = ps[(t, half)]
                gFIG[half] = gp.tile([H, 3 * NW], _f16, tag="gFIG",
                                     name=f"gFIG{t}_{half}")
                nc.scalar.activation(gFIG[half][:], psFIG[:], AF.Sigmoid)

            def evict_o(half):
                _, psO = ps[(t, half)]
                gO[half] = gp.tile([H, NW], _f16, tag="gO",
                                   name=f"gO{t}_{half}")
                nc.scalar.activation(gO[half][:], psO[:], AF.Sigmoid)

            def cell(half):
                """c = f*c_prev + i*(2*sg-1): tg via 4x tensor_scalar, the
                rest 2x tensor_tensor, all fp16 SBUF."""
                cs = slice(half * NW, (half + 1) * NW)
                f_s = gFIG[half][:, 0:NW]
                i_s = gFIG[half][:, NW : 2 * NW]
                sg = gFIG[half][:, 2 * NW : 3 * NW]
                tg = tp.tile([H, NW], _f16, tag="tg", name=f"tg{t}_{half}")
                nc.vector.tensor_scalar(tg[:], sg, 2.0, -1.0, ALU.mult,
                                        ALU.add)
                t1 = tp.tile([H, NW], _f16, tag="t1", name=f"t1_{t}_{half}")
                nc.vector.tensor_mul(t1[:], i_s, tg[:])
                t2 = tp.tile([H, NW], _f16, tag="t2", name=f"t2_{t}_{half}")
                nc.vector.tensor_mul(t2[:], f_s, c_prev[:, cs])
                nc.vector.tensor_add(c_new[:, cs], t1[:], t2[:])

            def tanh_c(half):
                cs = slice(half * NW, (half + 1) * NW)
                ct[half] = tp.tile([H, NW], _f16, tag="ct",
                                   name=f"ct{t}_{half}")
                nc.scalar.activation(ct[half][:], c_new[:, cs], AF.Tanh)

            def h_mul(half):
                """h = o * tanh(c) in two 256-col chunks so the next step's
                f,i matmuls can start on the first chunk early."""
                hw_ = NW // 2
                for q in range(2):
                    qs = slice(half * NW + q * hw_, half * NW + (q + 1) * hw_)
                    qq = slice(q * hw_, (q + 1) * hw_)
                    nc.vector.tensor_mul(h_new[:, qs], gO[half][:, qq],
                                         ct[half][:, qq])

            # ACT order: sFIG0 sFIG1 sO0 ct0 sO1 ct1. ct1 last so cell1
            # (which can only start once sFIG1 lands) makes its slot; sO1
            # waits ~16ns for the single O bank freed by sO0.
            if t > 0:
                psFIG0, psO0 = ps[(t, 0)]
                h_mms_fig(t, 0, psFIG0)
                h_mm_o(t, 0, psO0)
            evict_fig(0)
            cell(0)
            if t > 0:
                psFIG1, psO1 = ps[(t, 1)]
                h_mms_fig(t, 1, psFIG1)
                h_mm_o(t, 1, psO1)
            evict_fig(1)
            evict_o(0)
            tanh_c(0)
            cell(1)
            h_mul(0)
            evict_o(1)
            tanh_c(1)
            h_mul(1)
            ps.pop((t, 0))
            ps.pop((t, 1))
            if t + 1 < P:
                zt = zp.tile([I, BL], _f32r, tag="z", name=f"z{t + 1}")
                nc.sync.dma_start(zt[:], zt_d[t + 1, :, :])
                z_mms(t + 1, zt, ps)
            if prev is not None:
                emit_po(prev[0], prev[1])
            prev = (t, h_new)
            h_prev, c_prev = h_new, c_new
        emit_po(prev[0], prev[1])

    nc.compile()
    return nc


def _get_nc():
    if "nc" not in _CACHE:
        _CACHE["nc"] = _build()
    return _CACHE["nc"]


# gate-row permutation: PyTorch order [i,f,g,o] -> kernel order [f,i,g,o]
_PERM = np.concatenate(
    [np.arange(H, 2 * H), np.arange(0, H), np.arange(2 * H, 3 * H),
     np.arange(3 * H, 4 * H)]
)


def _prep_in_maps(x, z, h0, c0, W_ih, W_hh, b_ih, b_hh, W_out, b_out):
    f = np.float32
    Wihp = W_ih[_PERM]                                   # (512, 64)
    Whhp = W_hh[_PERM]                                   # (512, 128)
    Whfp = Whhp + Wihp[:, 0:1] @ W_out                   # fold out-projection
    b0 = (b_ih + b_hh)[_PERM].astype(f)
    bf = (b0 + Wihp[:, 0] * b_out[0]).astype(f)

    # G-gate rows (kernel order [f,i,g,o] -> rows 2H:3H) pre-scaled by 2:
    # the fused sigmoid eviction computes sg = sigmoid(2g) and the cell
    # applies tanh(g) = 2*sg - 1.
    gsc = np.ones((G4,), dtype=f)
    gsc[2 * H : 3 * H] = 2.0

    wz0t = (np.concatenate([Wihp.T, b0[None, :]], axis=0)
            * gsc[None, :]).astype(f)                    # (65, 512)
    wzft = (np.concatenate([Wihp.T, bf[None, :]], axis=0)
            * gsc[None, :]).astype(f)                    # (65, 512)
    whh0t = np.ascontiguousarray(Whhp.T * gsc[None, :], dtype=f)  # (128, 512)
    whhft = np.ascontiguousarray(Whfp.T * gsc[None, :], dtype=f)  # (128, 512)
    woutt = np.zeros((H, H), dtype=f)
    woutt[:, 63] = W_out[0]

    in_maps = []
    for m in range(NCORES):
        sl = slice(m * BL, (m + 1) * BL)
        z_aug = np.empty((P, I, BL), dtype=f)
        z_aug[:, 0, :] = 0.0
        z_aug[0, 0, :] = x[sl, -1, 0]
        z_aug[:, 1:-1, :] = np.transpose(z[sl, T - P :, :], (1, 2, 0))
        z_aug[:, -1, :] = 1.0
        wh0h = np.concatenate([whh0t, h0[0, sl, :].T], axis=1)   # (128, 1536)
        wrest = np.concatenate(
            [whhft, woutt, c0[0, sl, :].T], axis=1)              # (128, 1664)
        in_maps.append(
            {
                "zt": np.ascontiguousarray(z_aug),
                "wzt": np.ascontiguousarray(
                    np.concatenate([wz0t, wzft], axis=1)),       # (65, 1024)
                "wh0ht": np.ascontiguousarray(wh0h).astype(np.float16),
                "wrestt": np.ascontiguousarray(wrest).astype(np.float16),
            }
        )
    return in_maps


def run_on_cores(inputs: dict, **spmd_kwargs):
    """Build + run; returns (full_output, BassKernelResults)."""
    inputs = {k: np.asarray(v, dtype=np.float32) for k, v in inputs.items()}
    nc = _get_nc()
    in_maps = _prep_in_maps(**inputs)
    res = run_bass_kernel_spmd(nc, in_maps, core_ids=list(range(NCORES)), **spmd_kwargs)
    outs = np.concatenate(
        [r["out"].T for r in res.results], axis=0
    )  # (8192, 64)
    outs = outs + np.float32(inputs["b_out"][0])
    return outs[:, :, None].astype(np.float32), res


def kernel(**inputs) -> np.ndarray:
    out, _ = run_on_cores(inputs)
    return out
